# revision 1
# baseline (speedup 1.0000x reference)
"""Cross-modal attention block on 8 Trainium2 NeuronCores.

Sharding: core = 2*b + g  ->  batch b (4-way data parallel) x head-group g
(2-way tensor parallel over 16 heads -> 8 heads/core).  Each core:
  rownorm(x[b]) -> PE transpose -> q projection (ternary weights, gamma/beta
  folded) ; kT/v projections from pre-transposed context ; per-head
  scoresT = k~^T q~ (K=64 matmuls), exp on ScalarE, unnormalized attn-out
  with an appended ones-row producing softmax denominators in the same
  matmul ; normalize ; out-proj partial.  Host sums the two partials per
  batch + residual + folded biases.

All matmuls run in float32r (fp32 data, ~13-bit-mantissa PE path, 1 cyc/row).
"""

import os

import numpy as np

import concourse.bass as bass
import concourse.mybir as mybir
import concourse.tile as tile
from concourse import bacc
from concourse.bass_utils import run_bass_kernel_spmd
from concourse.masks import make_identity

FP = mybir.dt.float32
FPR = mybir.dt.float32r
BF = mybir.dt.bfloat16

B, T, TC, C = 4, 1024, 2048, 1024
H, HD = 16, 64
HL = 8           # heads per core
CL = HL * HD     # 512 local channels
SCALE = HD ** -0.5
LN_EPS = 1e-5
Q_EPS = 1e-5
P = 128
NCORES = 8

last_exec_time_ns = None


def _build_nc():
    nc = bacc.Bacc(None, target_bir_lowering=False, debug=False)

    x_d = nc.dram_tensor("x", [T // P, P, C], FP, kind="ExternalInput")
    ctxT_d = nc.dram_tensor("ctxT", [P, C // P, TC], FPR, kind="ExternalInput")
    wqT_d = nc.dram_tensor("wqT", [P, C // P, CL], FPR, kind="ExternalInput")
    wkT_d = nc.dram_tensor("wkT", [P, C // P, CL], FPR, kind="ExternalInput")
    wvT_d = nc.dram_tensor("wvT", [P, C // P, CL], FPR, kind="ExternalInput")
    woT_d = nc.dram_tensor("woT", [P, CL // P, C], FPR, kind="ExternalInput")
    cb_d = nc.dram_tensor("cb", [P, 9], FP, kind="ExternalInput")
    part_d = nc.dram_tensor("partial", [C // P, P, T], FP, kind="ExternalOutput")

    NT = T // P            # 8 query-row tiles
    NKC = C // P           # 8 contraction chunks over C
    NJ = TC // P           # 16 context chunks
    NM = CL // P           # 4 local d-chunks
    NH = T // 512          # 2 query halves

    with tile.TileContext(nc) as tc:
        with (
            tc.tile_pool(name="const", bufs=1) as cpool,
            tc.tile_pool(name="acts", bufs=1) as apool,
        ):
            ident_f = cpool.tile([P, P], FP)
            make_identity(nc, ident_f[:])
            ident = cpool.tile([P, P], FPR)
            nc.vector.tensor_copy(ident[:], ident_f[:])
            ones_f = cpool.tile([P, P], FP)
            nc.vector.memset(ones_f[:], 1.0)
            ones_r = cpool.tile([P, P], FPR)
            nc.vector.tensor_copy(ones_r[:], ones_f[:])
            ones_b = cpool.tile([P, P], BF)
            nc.vector.tensor_copy(ones_b[:], ones_f[:])
            cb = cpool.tile([P, 9], FP)
            nc.sync.dma_start(cb[:], cb_d[:])
            eps = cpool.tile([P, 1], FP)
            nc.vector.memset(eps[:], LN_EPS)

            qT = apool.tile([P, NM, T], FPR, tag="qT")
            kT = apool.tile([P, NM, TC], FPR, tag="kT")
            vv = apool.tile([P, NJ, HL * (HD + 1)], FPR, tag="vv")

            # ones column of v' (denominator rows), written once
            nc.vector.tensor_copy(
                vv[:].rearrange("p j (h c) -> p (j h) c", c=HD + 1)[:, :, HD : HD + 1],
                ones_r[:, 0 : NJ * HL][:, :, None],
            )

            with (
                tc.tile_pool(name="psmm", bufs=3, space="PSUM") as psmm,
                tc.tile_pool(name="ctx", bufs=1) as ctxpool,
            ):
                # ---- phase A1: rownorm + transpose + q projection ----
                with (
                    tc.tile_pool(name="xrn", bufs=3) as xpool,
                    tc.tile_pool(name="xst", bufs=6) as spool,
                    tc.tile_pool(name="rnt", bufs=1) as rpool,
                    tc.tile_pool(name="wqp", bufs=1) as wqpool,
                    tc.tile_pool(name="pstr", bufs=2, space="PSUM") as pstr,
                ):
                    xts = {}
                    for t in range(2):
                        xts[t] = xpool.tile([P, C], FP, tag="xt", name=f"xt{t}")
                        nc.sync.dma_start(xts[t][:], x_d[t])
                    wq = wqpool.tile([P, NKC, CL], FPR, tag="wq")
                    nc.sync.dma_start(wq[:], wqT_d[:])
                    ctxT0 = ctxpool.tile([P, NKC, TC // 2], FPR, tag="ctxT", name="ctxT0")
                    for k in range(NKC):
                        nc.sync.dma_start(ctxT0[:, k, :], ctxT_d[:, k, 0 : TC // 2])
                    rnT = rpool.tile([P, NKC, T], FPR, tag="rnT")
                    for t in range(NT):
                        if t < 2:
                            xt = xts[t]
                        else:
                            xt = xpool.tile([P, C], FP, tag="xt", name=f"xt{t}")
                            nc.sync.dma_start(xt[:], x_d[t])
                        nmu = spool.tile([P, 1], FP, tag="nmu")
                        nc.vector.reduce_sum(nmu[:], xt[:], axis=mybir.AxisListType.X)
                        nc.scalar.mul(nmu[:], nmu[:], -1.0 / C)
                        rn = xpool.tile([P, C], FPR, tag="rn")
                        ex2 = spool.tile([P, 1], FP, tag="ex2")
                        nc.scalar.activation(
                            rn[:], xt[:], mybir.ActivationFunctionType.Square,
                            accum_out=ex2[:],
                        )
                        var = spool.tile([P, 1], FP, tag="var")
                        nc.scalar.mul(ex2[:], ex2[:], 1.0 / C)
                        mu2 = spool.tile([P, 1], FP, tag="mu2")
                        nc.vector.tensor_mul(mu2[:], nmu[:], nmu[:])
                        nc.vector.tensor_sub(var[:], ex2[:], mu2[:])
                        std = spool.tile([P, 1], FP, tag="std")
                        nc.scalar.activation(
                            std[:], var[:], mybir.ActivationFunctionType.Sqrt,
                            bias=eps[:],
                        )
                        inv = spool.tile([P, 1], FP, tag="inv")
                        nc.vector.reciprocal(inv[:], std[:])
                        nc.vector.scalar_tensor_tensor(
                            out=rn[:], in0=xt[:], scalar=nmu[:],
                            in1=inv[:].to_broadcast((P, C)),
                            op0=mybir.AluOpType.add, op1=mybir.AluOpType.mult,
                        )
                        for c in range(NKC):
                            pt = pstr.tile([P, P], FP, tag="ptr")
                            nc.tensor.transpose(
                                pt[:].bitcast(FPR), rn[:, c * P : (c + 1) * P],
                                ident[:],
                            )
                            nc.scalar.copy(rnT[:, c, t * P : (t + 1) * P], pt[:])

                    # ---- q projection: qT[m] += wq[k,m]^T @ rnT[k] ----
                    for m in range(NM):
                        for n in range(2):
                            ps = psmm.tile([P, 512], FP, tag="mm")
                            for k in range(NKC):
                                nc.tensor.matmul(
                                    ps[:],
                                    wq[:, k, m * P : (m + 1) * P],
                                    rnT[:, k, n * 512 : (n + 1) * 512],
                                    start=(k == 0), stop=(k == NKC - 1),
                                )
                            nc.vector.tensor_scalar(
                                out=qT[:, m, n * 512 : (n + 1) * 512], in0=ps[:],
                                scalar1=cb[:, m : m + 1], scalar2=cb[:, 8:9],
                                op0=mybir.AluOpType.add, op1=mybir.AluOpType.mult,
                            )

                # ---- k/v projections, context streamed in halves ----
                with (
                    tc.tile_pool(name="wkv", bufs=1) as wpool,
                ):
                    wk = wpool.tile([P, NKC, CL], FPR, tag="wk")
                    wv = wpool.tile([P, NKC, CL], FPR, tag="wv")
                    nc.sync.dma_start(wk[:], wkT_d[:])
                    nc.sync.dma_start(wv[:], wvT_d[:])
                    for ch in range(2):
                        if ch == 0:
                            ctxT = ctxT0
                        else:
                            ctxT = ctxpool.tile([P, NKC, TC // 2], FPR, tag="ctxT", name="ctxT1")
                            for k in range(NKC):
                                nc.sync.dma_start(
                                    ctxT[:, k, :], ctxT_d[:, k, TC // 2 : TC],
                                )
                        # k projection for this context half
                        for m in range(NM):
                            for n2 in range(2):
                                n = 2 * ch + n2
                                ps = psmm.tile([P, 512], FP, tag="mm")
                                for k in range(NKC):
                                    nc.tensor.matmul(
                                        ps[:],
                                        wk[:, k, m * P : (m + 1) * P],
                                        ctxT[:, k, n2 * 512 : (n2 + 1) * 512],
                                        start=(k == 0), stop=(k == NKC - 1),
                                    )
                                nc.vector.tensor_scalar_add(
                                    kT[:, m, n * 512 : (n + 1) * 512], ps[:],
                                    cb[:, 4 + m : 5 + m],
                                )
                        # v projection for this context half
                        for jj in range(NJ // 2):
                            j = ch * (NJ // 2) + jj
                            ps = psmm.tile([P, 512], FP, tag="mm")
                            for k in range(NKC):
                                nc.tensor.matmul(
                                    ps[:],
                                    ctxT[:, k, jj * P : (jj + 1) * P],
                                    wv[:, k, :],
                                    start=(k == 0), stop=(k == NKC - 1),
                                )
                            nc.vector.tensor_copy(
                                vv[:, j, :].rearrange("p (h c) -> p h c", c=HD + 1)[:, :, 0:HD],
                                ps[:].rearrange("p (h c) -> p h c", c=HD),
                            )

            # ---- attention + out-proj ----
            with (
                tc.tile_pool(name="wo", bufs=1) as wopool,
                tc.tile_pool(name="att", bufs=1) as attpool,
                tc.tile_pool(name="exp", bufs=15) as epool,
                tc.tile_pool(name="nrm", bufs=4) as npool,
            ):
                wo = wopool.tile([P, NM, C], FPR, tag="wo")
                nc.sync.dma_start(wo[:], woT_d[:])
                attnT = attpool.tile([P, NM, T], FPR, tag="attnT")

                with (
                    tc.tile_pool(name="pssc", bufs=2, space="PSUM") as pssc,
                    tc.tile_pool(name="psat", bufs=4, space="PSUM") as psat,
                    tc.tile_pool(name="scrd", bufs=4, space="DRAM") as dpool,
                ):
                    JB = 8                      # context chunks per mode-batch
                    for i in range(HL // 2):    # head pairs (2i, 2i+1)
                        ph = {}
                        for hh in range(2):
                            for H in range(NH):
                                ph[hh, H] = psat.tile(
                                    [HD + 1, 512], FP, tag="ph", name=f"ph_{hh}_{H}",
                                )
                        for jb in range(NJ // JB):
                            ets = {}
                            # scores (64-row PE tiling) + exp, batched
                            for jj in range(JB):
                                j = JB * jb + jj
                                for hh in range(2):
                                    prow = 64 * hh
                                    psc = pssc.tile(
                                        [P, T], FP, tag="sc", name=f"sc_{jj}_{hh}",
                                    )
                                    for H in range(NH):
                                        nc.tensor.matmul(
                                            psc[:, H * 512 : (H + 1) * 512],
                                            kT[prow : prow + HD, i, j * P : (j + 1) * P],
                                            qT[prow : prow + HD, i, H * 512 : (H + 1) * 512],
                                            start=True, stop=True,
                                        )
                                    et = epool.tile([P, T], FPR, tag="et", name=f"et_{jj}_{hh}")
                                    nc.scalar.activation(
                                        et[:], psc[:], mybir.ActivationFunctionType.Exp,
                                    )
                                    ets[jj, hh] = et
                            # unnormalized attn-out (128-row tiling), batched
                            for jj in range(JB):
                                j = JB * jb + jj
                                for hh in range(2):
                                    h = 2 * i + hh
                                    for H in range(NH):
                                        nc.tensor.matmul(
                                            ph[hh, H][:],
                                            vv[:, j, h * (HD + 1) : (h + 1) * (HD + 1)],
                                            ets[jj, hh][:, H * 512 : (H + 1) * 512],
                                            start=(j == 0), stop=(j == NJ - 1),
                                        )
                        # normalize via DRAM-roundtrip partition broadcast (no PE)
                        for hh in range(2):
                            prow = 64 * hh
                            for H in range(NH):
                                au = npool.tile([HD + 1, 512], FP, tag="au",
                                                name=f"au_{hh}_{H}")
                                nc.vector.tensor_copy(au[:], ph[hh, H][:])
                                sr = npool.tile([1, 512], FP, tag="sr")
                                nc.vector.reciprocal(sr[:], au[HD : HD + 1, :])
                                rb = npool.tile([HD, 512], FP, tag="rb")
                                if i == HL // 2 - 1:
                                    nc.gpsimd.partition_broadcast(rb[:], sr[:])
                                else:
                                    sd = dpool.tile([1, 512], FP, tag="sd")
                                    nc.sync.dma_start(sd[:], sr[:])
                                    nc.sync.dma_start(rb[:], sd[:].to_broadcast((HD, 512)))
                                nc.vector.tensor_mul(
                                    attnT[prow : prow + HD, i, H * 512 : (H + 1) * 512],
                                    au[0:HD, :], rb[:],
                                )

                # out-proj partials
                with tc.tile_pool(name="psoc", bufs=3, space="PSUM") as psoc:
                    with tc.tile_pool(name="oev", bufs=3) as opool:
                        for H in range(NH):
                            hs = slice(H * 512, (H + 1) * 512)
                            for m in range(C // P):
                                po = psoc.tile([P, 512], FP, tag="oc")
                                for k2 in range(NM):
                                    nc.tensor.matmul(
                                        po[:],
                                        wo[:, k2, m * P : (m + 1) * P],
                                        attnT[:, k2, hs],
                                        start=(k2 == 0), stop=(k2 == NM - 1),
                                    )
                                ot = opool.tile([P, 512], FP, tag="ot")
                                nc.vector.tensor_copy(ot[:], po[:])
                                nc.sync.dma_start(part_d[m, :, hs], ot[:])

    nc.finalize()
    return nc


_NC_CACHE = {}


def _get_nc():
    if "nc" not in _NC_CACHE:
        _NC_CACHE["nc"] = _build_nc()
    return _NC_CACHE["nc"]


def _quant(w):
    g = np.float32(np.mean(np.abs(w), dtype=np.float64))
    t = np.clip(np.rint(w / (g + np.float32(Q_EPS))), -1.0, 1.0).astype(np.float32)
    return t, g


def _pack_kp(a):
    # [K, M] -> [P, K//P, M] (partition-major chunks)
    k, m = a.shape
    return np.ascontiguousarray(a.reshape(k // P, P, m).transpose(1, 0, 2))


def kernel(**inputs):
    global last_exec_time_ns
    x = np.asarray(inputs["x"], dtype=np.float32)
    ctx = np.asarray(inputs["context"], dtype=np.float32)
    Wq = np.asarray(inputs["Wq"], dtype=np.float32)
    Wk = np.asarray(inputs["Wk"], dtype=np.float32)
    Wv = np.asarray(inputs["Wv"], dtype=np.float32)
    Wo = np.asarray(inputs["Wo"], dtype=np.float32)
    bq = np.asarray(inputs["bq"], dtype=np.float32)
    bk = np.asarray(inputs["bk"], dtype=np.float32)
    bv = np.asarray(inputs["bv"], dtype=np.float32)
    bo = np.asarray(inputs["bo"], dtype=np.float32)
    g_ln = np.asarray(inputs["ln_gamma"], dtype=np.float32)
    b_ln = np.asarray(inputs["ln_beta"], dtype=np.float32)

    Tq, gq = _quant(Wq)
    Tk, gk = _quant(Wk)
    Tv, gv = _quant(Wv)
    To, go = _quant(Wo)

    qb_full = (bq + b_ln @ (gq * Tq).T) / gq          # [C]
    scale = np.float32(gq * gk * SCALE)
    host_bias = bo + bv @ (go * To).T                 # [C]

    in_maps = []
    for core in range(NCORES):
        b = core // 2
        g = core % 2
        rows = slice(CL * g, CL * (g + 1))
        wqT = _pack_kp((Tq[rows] * g_ln[None, :]).T)  # [P, 8, 512]
        wkT = _pack_kp(Tk[rows].T)
        wvT = _pack_kp(Tv[rows].T)
        woT = _pack_kp((To[:, rows] * (go * gv)).T)   # [P, 4, 1024]
        cbm = np.zeros((P, 9), dtype=np.float32)
        cbm[:, 0:4] = qb_full[rows].reshape(4, P).T
        cbm[:, 4:8] = (bk[rows] / gk).reshape(4, P).T
        cbm[:, 8] = scale
        in_maps.append({
            "x": np.ascontiguousarray(x[b].reshape(T // P, P, C)),
            "ctxT": _pack_kp(np.ascontiguousarray(ctx[b].T)),
            "wqT": wqT, "wkT": wkT, "wvT": wvT, "woT": woT,
            "cb": cbm,
        })

    nc = _get_nc()
    trace = os.environ.get("KERNEL_TRACE", "0") == "1"
    res = run_bass_kernel_spmd(nc, in_maps, list(range(NCORES)), trace=trace)
    last_exec_time_ns = res.exec_time_ns

    out = np.empty((B, T, C), dtype=np.float32)
    for b in range(B):
        p0 = res.results[2 * b]["partial"].reshape(C, T)
        p1 = res.results[2 * b + 1]["partial"].reshape(C, T)
        out[b] = x[b] + p0.T + p1.T + host_bias[None, :]
    return out



# revision 11
# speedup vs baseline: 1.6819x; 1.6819x over previous
"""Cross-modal attention block on 8 Trainium2 NeuronCores.

Sharding: core = 2*b + g  ->  batch b (4-way data parallel) x head-group g
(2-way tensor parallel over 16 heads -> 8 heads/core).  Each core:
  rownorm(x[b]) -> PE transpose -> q projection (ternary weights, gamma/beta
  folded) ; kT/v projections from pre-transposed context ; per-head
  scoresT = k~^T q~ ; exp split between ScalarE (exact) and VectorE
  (Schraudolph bit-trick into bf16) ; unnormalized attn-out with an appended
  ones-row producing softmax denominators in the same matmul ; deferred
  batch normalize (reciprocal_approx_fast + select-matmul broadcast) ;
  out-proj partial.  Host sums the two partials per batch + residual +
  folded biases.

All matmuls are full 128x128-mode bf16 (scores use zero-padded K so the PE
never enters a tiled mode, which measures as HAM-throttled 1.2 GHz).
"""

import os

import ml_dtypes
import numpy as np

import concourse.bass as bass
import concourse.mybir as mybir
import concourse.tile as tile
from concourse import bacc
from concourse.bass_utils import run_bass_kernel_spmd
from concourse.masks import make_identity

FP = mybir.dt.float32
FPR = mybir.dt.float32r
BF = mybir.dt.bfloat16
I16 = mybir.dt.int16

B, T, TC, C = 4, 1024, 2048, 1024
H, HD = 16, 64
HL = 8           # heads per core
CL = HL * HD     # 512 local channels
SCALE = HD ** -0.5
LN_EPS = 1e-5
Q_EPS = 1e-5
P = 128
NCORES = 8

NT = T // P      # 8 query-row tiles
NKC = C // P     # 8 contraction chunks over C
NJ = TC // P     # 16 context chunks
NM = CL // P     # 4 local d-chunks

# Schraudolph fast-exp into bf16 bit pattern via int16:
#   i16 = trunc(x * EXP_A + EXP_B); bf16 = bits(i16)
# max rel err ~3.3% over x in [-10, 8]; scores*scale stay well inside.
EXP_A = float(np.float32(128.0 / np.log(2.0)))
EXP_B = float(np.float32(16256.0 - 5.1))

last_exec_time_ns = None


def _build_nc():
    nc = bacc.Bacc(None, target_bir_lowering=False, debug=False)

    x_d = nc.dram_tensor("x", [NT, P, C], BF, kind="ExternalInput")
    ctxT_d = nc.dram_tensor("ctxT", [P, NKC, TC], BF, kind="ExternalInput")
    wqT_d = nc.dram_tensor("wqT", [P, NKC, CL], BF, kind="ExternalInput")
    wkT_d = nc.dram_tensor("wkT", [P, NKC, CL], BF, kind="ExternalInput")
    wvT_d = nc.dram_tensor("wvT", [P, NKC, CL], BF, kind="ExternalInput")
    woT_d = nc.dram_tensor("woT", [P, NM, C], BF, kind="ExternalInput")
    cb_d = nc.dram_tensor("cb", [P, 9], FP, kind="ExternalInput")
    sel_d = nc.dram_tensor("sel", [P, NM, P], BF, kind="ExternalInput")
    part_d = nc.dram_tensor("partial", [C // P, P, T], BF, kind="ExternalOutput")

    with tile.TileContext(nc) as tc:
        with (
            tc.tile_pool(name="const", bufs=1) as cpool,
            tc.tile_pool(name="acts", bufs=1) as apool,
        ):
            ident_f = cpool.tile([P, P], FP)
            make_identity(nc, ident_f[:])
            ident = cpool.tile([P, P], BF)
            nc.vector.tensor_copy(ident[:], ident_f[:])
            cb = cpool.tile([P, 9], FP)
            nc.sync.dma_start(cb[:], cb_d[:])
            sel = cpool.tile([P, NM, P], BF)
            nc.sync.dma_start(sel[:], sel_d[:])
            eps = cpool.tile([P, 1], FP)
            nc.vector.memset(eps[:], LN_EPS)

            rnT = apool.tile([P, NKC, T], BF, tag="rnT")
            qT = apool.tile([P, NM, T], BF, tag="qT")
            # Scores stationaries, zero-padded so every matmul is full K=128:
            # kTa rows 0-63 = head-a k rows (rows 64-127 zero), kTb vice versa.
            kTa = apool.tile([P, NM, TC], BF, tag="kTa")
            kTb = apool.tile([P, NM, TC], BF, tag="kTb")
            vv = apool.tile([P, NJ, HL * (HD + 1)], BF, tag="vv")
            attnU = apool.tile([P, NM, T], BF, tag="attnU")
            attnT = apool.tile([P, NM, T], BF, tag="attnT")
            # denominator rows live at 32-aligned partitions (BIR requires
            # engine APs to start on partition multiples of 32):
            # row(i, hh) -> partition 32*(2*(i%2)+hh), column half i//2
            denp = apool.tile([P, 2 * T], FP, tag="denp")
            recipp = apool.tile([P, 2 * T], FP, tag="recipp")
            recipb = apool.tile([P, 2 * T], BF, tag="recipb")

            # one-time zero/one fills (DVE, overlapped with initial DMAs)
            nc.vector.memset(kTa[64:128, :, :], 0.0)
            nc.vector.memset(kTb[0:64, :, :], 0.0)
            nc.vector.memset(denp[:], 1.0)
            # ones column of v' (denominator rows)
            nc.vector.memset(
                vv[:].rearrange("p j (h c) -> p (j h) c", c=HD + 1)[:, :, HD : HD + 1],
                1.0,
            )
            # preload the Exp activation table before the attention phase
            dummy = cpool.tile([P, 1], BF)

            with (
                tc.tile_pool(name="psmm", bufs=2, space="PSUM") as psmm,
                tc.tile_pool(name="ctx", bufs=2) as ctxpool,
            ):
                # ---- phase A1: rownorm + transpose ----
                with (
                    tc.tile_pool(name="xrn", bufs=3) as xpool,
                    tc.tile_pool(name="xst", bufs=6) as spool,
                    tc.tile_pool(name="sqp", bufs=2) as sqpool,
                    tc.tile_pool(name="wqp", bufs=1) as wqpool,
                    tc.tile_pool(name="pstr", bufs=2, space="PSUM") as pstr,
                ):
                    xts = {}
                    for t in range(2):
                        xts[t] = xpool.tile([P, C], BF, tag="xt", name=f"xt{t}")
                        nc.sync.dma_start(xts[t][:], x_d[t])
                    wq = wqpool.tile([P, NKC, CL], BF, tag="wq")
                    nc.sync.dma_start(wq[:], wqT_d[:])
                    ctxT0 = ctxpool.tile([P, NKC, TC // 2], BF, tag="ctxT", name="ctxT0")
                    for k in range(NKC):
                        nc.sync.dma_start(ctxT0[:, k, :], ctxT_d[:, k, 0 : TC // 2])
                    for t in range(NT):
                        if t < 2:
                            xt = xts[t]
                        else:
                            xt = xpool.tile([P, C], BF, tag="xt", name=f"xt{t}")
                            nc.sync.dma_start(xt[:], x_d[t])
                        nmu = spool.tile([P, 1], FP, tag="nmu")
                        nc.vector.reduce_sum(nmu[:], xt[:], axis=mybir.AxisListType.X)
                        nc.scalar.mul(nmu[:], nmu[:], -1.0 / C)
                        sq = sqpool.tile([P, C], BF, tag="sq")
                        ex2 = spool.tile([P, 1], FP, tag="ex2")
                        nc.scalar.activation(
                            sq[:], xt[:], mybir.ActivationFunctionType.Square,
                            accum_out=ex2[:],
                        )
                        var = spool.tile([P, 1], FP, tag="var")
                        nc.scalar.mul(ex2[:], ex2[:], 1.0 / C)
                        mu2 = spool.tile([P, 1], FP, tag="mu2")
                        nc.vector.tensor_mul(mu2[:], nmu[:], nmu[:])
                        nc.vector.tensor_sub(var[:], ex2[:], mu2[:])
                        std = spool.tile([P, 1], FP, tag="std")
                        nc.scalar.activation(
                            std[:], var[:], mybir.ActivationFunctionType.Sqrt,
                            bias=eps[:],
                        )
                        inv = spool.tile([P, 1], FP, tag="inv")
                        nc.vector.reciprocal(inv[:], std[:])
                        rn = xpool.tile([P, C], BF, tag="rn")
                        nc.vector.scalar_tensor_tensor(
                            out=rn[:], in0=xt[:], scalar=nmu[:],
                            in1=inv[:].to_broadcast((P, C)),
                            op0=mybir.AluOpType.add, op1=mybir.AluOpType.mult,
                        )
                        # 4 transposes per PSUM tile, one evacuation copy each
                        for c4 in range(2):
                            pt = pstr.tile([P, 512], BF, tag="ptr")
                            for cc in range(4):
                                c = 4 * c4 + cc
                                nc.tensor.transpose(
                                    pt[:, cc * P : (cc + 1) * P],
                                    rn[:, c * P : (c + 1) * P],
                                    ident[:],
                                )
                            nc.vector.tensor_copy(
                                rnT[:, 4 * c4 : 4 * c4 + 4, t * P : (t + 1) * P],
                                pt[:].rearrange("p (c q) -> p c q", q=P),
                            )

                    # ---- q projection: qT[m] += wq[k,m]^T @ rnT[k] ----
                    for m in range(NM):
                        ps = psmm.tile([P, T], FP, tag="mm")
                        for n in range(2):
                            ns = slice(n * 512, (n + 1) * 512)
                            for k in range(NKC):
                                nc.tensor.matmul(
                                    ps[:, ns],
                                    wq[:, k, m * P : (m + 1) * P],
                                    rnT[:, k, ns],
                                    start=(k == 0), stop=(k == NKC - 1),
                                )
                        nc.vector.tensor_scalar(
                            out=qT[:, m, :], in0=ps[:],
                            scalar1=cb[:, m : m + 1], scalar2=cb[:, 8:9],
                            op0=mybir.AluOpType.add, op1=mybir.AluOpType.mult,
                        )
                    # preload exp table after the last Sqrt
                    nc.scalar.activation(
                        dummy[:], eps[:], mybir.ActivationFunctionType.Exp,
                    )

                # ---- k/v projections, context streamed in halves ----
                with (
                    tc.tile_pool(name="wkv", bufs=1) as wpool,
                ):
                    wk = wpool.tile([P, NKC, CL], BF, tag="wk")
                    wv = wpool.tile([P, NKC, CL], BF, tag="wv")
                    nc.sync.dma_start(wk[:], wkT_d[:])
                    nc.sync.dma_start(wv[:], wvT_d[:])
                    for ch in range(2):
                        if ch == 0:
                            ctxT = ctxT0
                        else:
                            ctxT = ctxpool.tile([P, NKC, TC // 2], BF, tag="ctxT", name="ctxT1")
                            for k in range(NKC):
                                nc.sync.dma_start(
                                    ctxT[:, k, :], ctxT_d[:, k, TC // 2 : TC],
                                )
                        hs = slice(ch * (TC // 2), (ch + 1) * (TC // 2))
                        # k projection for this context half
                        for m in range(NM):
                            ps = psmm.tile([P, TC // 2], FP, tag="mm")
                            for n in range(2):
                                ns = slice(n * 512, (n + 1) * 512)
                                for k in range(NKC):
                                    nc.tensor.matmul(
                                        ps[:, ns],
                                        wk[:, k, m * P : (m + 1) * P],
                                        ctxT[:, k, ns],
                                        start=(k == 0), stop=(k == NKC - 1),
                                    )
                            nc.vector.tensor_scalar_add(
                                kTa[0:64, m, hs], ps[0:64, :], cb[0:64, 4 + m : 5 + m],
                            )
                            nc.vector.tensor_scalar_add(
                                kTb[64:128, m, hs], ps[64:128, :], cb[64:128, 4 + m : 5 + m],
                            )
                        # v projection for this context half
                        for jj in range(NJ // 2):
                            j = ch * (NJ // 2) + jj
                            ps = psmm.tile([P, CL], FP, tag="mmv")
                            for k in range(NKC):
                                nc.tensor.matmul(
                                    ps[:],
                                    ctxT[:, k, jj * P : (jj + 1) * P],
                                    wv[:, k, :],
                                    start=(k == 0), stop=(k == NKC - 1),
                                )
                            nc.vector.tensor_copy(
                                vv[:, j, :].rearrange("p (h c) -> p h c", c=HD + 1)[:, :, 0:HD],
                                ps[:].rearrange("p (h c) -> p h c", c=HD),
                            )

            # ---- attention: scores -> exp (ACT/DVE split) -> attn-out ----
            with (
                tc.tile_pool(name="wo", bufs=1) as wopool,
                tc.tile_pool(name="expa", bufs=3) as eapool,
                tc.tile_pool(name="expb", bufs=3) as ebpool,
            ):
                wo = wopool.tile([P, NM, C], BF, tag="wo")
                nc.sync.dma_start(wo[:], woT_d[:])

                with (
                    tc.tile_pool(name="pssc", bufs=2, space="PSUM") as pssc,
                    tc.tile_pool(name="psat", bufs=2, space="PSUM") as psat,
                ):
                    for i in range(NM):
                        ph = {}
                        for hh in range(2):
                            ph[hh] = psat.tile(
                                [HD + 1, T], FP, tag="ph", name=f"ph_{i}_{hh}",
                            )
                        ao_pending = []
                        for j in range(NJ):
                            js = slice(j * P, (j + 1) * P)
                            psc_a = pssc.tile([P, T], FP, tag="sc", name=f"sa{i}_{j}")
                            psc_b = pssc.tile([P, T], FP, tag="sc", name=f"sb{i}_{j}")
                            for n in range(2):
                                ns = slice(n * 512, (n + 1) * 512)
                                nc.tensor.matmul(psc_a[:, ns], kTa[:, i, js],
                                                 qT[:, i, ns],
                                                 start=True, stop=True)
                                nc.tensor.matmul(psc_b[:, ns], kTb[:, i, js],
                                                 qT[:, i, ns],
                                                 start=True, stop=True)
                            et_a = eapool.tile([P, T], BF, tag="eta", name=f"ea{i}_{j}")
                            nc.scalar.activation(
                                et_a[:], psc_a[:], mybir.ActivationFunctionType.Exp,
                            )
                            et_b = ebpool.tile([P, T], I16, tag="etb", name=f"eb{i}_{j}")
                            nc.vector.tensor_scalar(
                                out=et_b[:], in0=psc_b[:],
                                scalar1=EXP_A, scalar2=EXP_B,
                                op0=mybir.AluOpType.mult, op1=mybir.AluOpType.add,
                            )
                            ao_pending.append((j, et_a, et_b))
                            if j >= 1:
                                jp, pa, pb = ao_pending.pop(0)
                                vs = vv[:, jp, :].rearrange("p (h c) -> p h c", c=HD + 1)
                                for n in range(2):
                                    ns = slice(n * 512, (n + 1) * 512)
                                    nc.tensor.matmul(
                                        ph[0][:, ns], vs[:, 2 * i, :], pa[:, ns],
                                        start=(jp == 0), stop=(jp == NJ - 1),
                                    )
                                    nc.tensor.matmul(
                                        ph[1][:, ns], vs[:, 2 * i + 1, :],
                                        pb[:, ns].bitcast(BF),
                                        start=(jp == 0), stop=(jp == NJ - 1),
                                    )
                        jp, pa, pb = ao_pending.pop(0)
                        vs = vv[:, jp, :].rearrange("p (h c) -> p h c", c=HD + 1)
                        for n in range(2):
                            ns = slice(n * 512, (n + 1) * 512)
                            nc.tensor.matmul(ph[0][:, ns], vs[:, 2 * i, :], pa[:, ns],
                                             start=(jp == 0), stop=(jp == NJ - 1))
                            nc.tensor.matmul(ph[1][:, ns], vs[:, 2 * i + 1, :],
                                             pb[:, ns].bitcast(BF),
                                             start=(jp == 0), stop=(jp == NJ - 1))
                        # evacuate: data rows -> attnU (ACT), denom rows -> denp (DVE)
                        for hh in range(2):
                            nc.scalar.copy(
                                attnU[64 * hh : 64 * hh + 64, i, :], ph[hh][0:64, :],
                            )
                            dp = 32 * (2 * (i % 2) + hh)
                            dc = (i // 2) * T
                            nc.vector.tensor_copy(
                                denp[dp : dp + 1, dc : dc + T], ph[hh][64:65, :],
                            )

                # ---- deferred normalize ----
                with tc.tile_pool(name="psel", bufs=2, space="PSUM") as psel:
                    nc.vector.reciprocal_approx_fast(recipp[:], denp[:])
                    nc.vector.tensor_copy(recipb[:], recipp[:])
                    for i in range(NM):
                        dc = (i // 2) * T
                        rb = psel.tile([P, T], FP, tag="rb")
                        for n in range(2):
                            ns = slice(n * 512, (n + 1) * 512)
                            nc.tensor.matmul(
                                rb[:, ns], sel[:, i, :],
                                recipb[:, dc + n * 512 : dc + (n + 1) * 512],
                                start=True, stop=True,
                            )
                        nc.vector.tensor_mul(
                            attnT[:, i, :], attnU[:, i, :], rb[:],
                        )

                # ---- out-proj partials ----
                with tc.tile_pool(name="psoc", bufs=2, space="PSUM") as psoc:
                    with tc.tile_pool(name="oev", bufs=3) as opool:
                        for m in range(C // P):
                            po = psoc.tile([P, T], FP, tag="oc")
                            for n in range(2):
                                ns = slice(n * 512, (n + 1) * 512)
                                for k2 in range(NM):
                                    nc.tensor.matmul(
                                        po[:, ns],
                                        wo[:, k2, m * P : (m + 1) * P],
                                        attnT[:, k2, ns],
                                        start=(k2 == 0), stop=(k2 == NM - 1),
                                    )
                            ot = opool.tile([P, T], BF, tag="ot")
                            nc.vector.tensor_copy(ot[:], po[:])
                            nc.sync.dma_start(part_d[m], ot[:])

    nc.finalize()
    return nc


_NC_CACHE = {}


def _get_nc():
    if "nc" not in _NC_CACHE:
        _NC_CACHE["nc"] = _build_nc()
    return _NC_CACHE["nc"]


def _quant(w):
    g = np.float32(np.mean(np.abs(w), dtype=np.float64))
    t = np.clip(np.rint(w / (g + np.float32(Q_EPS))), -1.0, 1.0).astype(np.float32)
    return t, g


def _pack_kp(a):
    # [K, M] -> [P, K//P, M] (partition-major chunks)
    k, m = a.shape
    return np.ascontiguousarray(a.reshape(k // P, P, m).transpose(1, 0, 2))


def _bf(a):
    return np.ascontiguousarray(a.astype(ml_dtypes.bfloat16))


def kernel(**inputs):
    global last_exec_time_ns
    x = np.asarray(inputs["x"], dtype=np.float32)
    ctx = np.asarray(inputs["context"], dtype=np.float32)
    Wq = np.asarray(inputs["Wq"], dtype=np.float32)
    Wk = np.asarray(inputs["Wk"], dtype=np.float32)
    Wv = np.asarray(inputs["Wv"], dtype=np.float32)
    Wo = np.asarray(inputs["Wo"], dtype=np.float32)
    bq = np.asarray(inputs["bq"], dtype=np.float32)
    bk = np.asarray(inputs["bk"], dtype=np.float32)
    bv = np.asarray(inputs["bv"], dtype=np.float32)
    bo = np.asarray(inputs["bo"], dtype=np.float32)
    g_ln = np.asarray(inputs["ln_gamma"], dtype=np.float32)
    b_ln = np.asarray(inputs["ln_beta"], dtype=np.float32)

    Tq, gq = _quant(Wq)
    Tk, gk = _quant(Wk)
    Tv, gv = _quant(Wv)
    To, go = _quant(Wo)

    qb_full = (bq + b_ln @ (gq * Tq).T) / gq          # [C]
    scale = np.float32(gq * gk * SCALE)
    host_bias = bo + bv @ (go * To).T                 # [C]

    # select matrices for the denominator broadcast: recipp partition
    # 32*(2*(i%2)+hh) feeds partitions [64*hh, 64*hh+64) of attnT chunk i
    selm = np.zeros((P, NM, P), dtype=np.float32)
    for i in range(NM):
        selm[32 * (2 * (i % 2)), i, 0:64] = 1.0
        selm[32 * (2 * (i % 2) + 1), i, 64:128] = 1.0

    in_maps = []
    for core in range(NCORES):
        b = core // 2
        g = core % 2
        rows = slice(CL * g, CL * (g + 1))
        wqT = _pack_kp((Tq[rows] * g_ln[None, :]).T)  # [P, 8, 512]
        wkT = _pack_kp(Tk[rows].T)
        wvT = _pack_kp(Tv[rows].T)
        woT = _pack_kp((To[:, rows] * (go * gv)).T)   # [P, 4, 1024]
        cbm = np.zeros((P, 9), dtype=np.float32)
        cbm[:, 0:4] = qb_full[rows].reshape(4, P).T
        cbm[:, 4:8] = (bk[rows] / gk).reshape(4, P).T
        cbm[:, 8] = scale
        in_maps.append({
            "x": _bf(x[b].reshape(T // P, P, C)),
            "ctxT": _bf(_pack_kp(np.ascontiguousarray(ctx[b].T))),
            "wqT": _bf(wqT), "wkT": _bf(wkT), "wvT": _bf(wvT), "woT": _bf(woT),
            "cb": cbm,
            "sel": _bf(selm),
        })

    nc = _get_nc()
    trace = os.environ.get("KERNEL_TRACE", "0") == "1"
    res = run_bass_kernel_spmd(nc, in_maps, list(range(NCORES)), trace=trace)
    last_exec_time_ns = res.exec_time_ns

    out = np.empty((B, T, C), dtype=np.float32)
    for b in range(B):
        p0 = res.results[2 * b]["partial"].astype(np.float32).reshape(C, T)
        p1 = res.results[2 * b + 1]["partial"].astype(np.float32).reshape(C, T)
        out[b] = x[b] + p0.T + p1.T + host_bias[None, :]
    return out


# revision 12
# speedup vs baseline: 1.7272x; 1.0269x over previous
"""Cross-modal attention block on 8 Trainium2 NeuronCores.

Sharding: core = 2*b + g  ->  batch b (4-way data parallel) x head-group g
(2-way tensor parallel over 16 heads -> 8 heads/core).  Each core:
  rownorm(x[b]) -> PE transpose -> q projection (ternary weights, gamma/beta
  folded) ; kT/v projections from pre-transposed context ; per-head
  scoresT = k~^T q~ ; exp split between ScalarE (exact) and VectorE
  (Schraudolph bit-trick into bf16) ; unnormalized attn-out with an appended
  ones-row producing softmax denominators in the same matmul ; deferred
  batch normalize (reciprocal_approx_fast + select-matmul broadcast) ;
  out-proj partial.  Host sums the two partials per batch + residual +
  folded biases.

All matmuls are full 128x128-mode bf16 (scores use zero-padded K so the PE
never enters a tiled mode, which measures as HAM-throttled 1.2 GHz).
"""

import os

import ml_dtypes
import numpy as np

import concourse.bass as bass
import concourse.mybir as mybir
import concourse.tile as tile
from concourse import bacc
from concourse.bass_utils import run_bass_kernel_spmd
from concourse.masks import make_identity

FP = mybir.dt.float32
FPR = mybir.dt.float32r
BF = mybir.dt.bfloat16
I16 = mybir.dt.int16

B, T, TC, C = 4, 1024, 2048, 1024
H, HD = 16, 64
HL = 8           # heads per core
CL = HL * HD     # 512 local channels
SCALE = HD ** -0.5
LN_EPS = 1e-5
Q_EPS = 1e-5
P = 128
NCORES = 8

NT = T // P      # 8 query-row tiles
NKC = C // P     # 8 contraction chunks over C
NJ = TC // P     # 16 context chunks
NM = CL // P     # 4 local d-chunks

# Schraudolph fast-exp into bf16 bit pattern via int16:
#   i16 = trunc(x * EXP_A + EXP_B); bf16 = bits(i16)
# max rel err ~3.3% over x in [-10, 8]; scores*scale stay well inside.
EXP_A = float(np.float32(128.0 / np.log(2.0)))
EXP_B = float(np.float32(16256.0 - 5.1))

last_exec_time_ns = None


def _build_nc():
    nc = bacc.Bacc(None, target_bir_lowering=False, debug=False)

    x_d = nc.dram_tensor("x", [NT, P, C], BF, kind="ExternalInput")
    ctxT_d = nc.dram_tensor("ctxT", [P, NKC, TC], BF, kind="ExternalInput")
    wqT_d = nc.dram_tensor("wqT", [P, NKC, CL], BF, kind="ExternalInput")
    wkT_d = nc.dram_tensor("wkT", [P, NKC, CL], BF, kind="ExternalInput")
    wvT_d = nc.dram_tensor("wvT", [P, NKC, CL], BF, kind="ExternalInput")
    woT_d = nc.dram_tensor("woT", [P, NM, C], BF, kind="ExternalInput")
    cb_d = nc.dram_tensor("cb", [P, 9], FP, kind="ExternalInput")
    sel_d = nc.dram_tensor("sel", [P, NM, P], BF, kind="ExternalInput")
    part_d = nc.dram_tensor("partial", [C // P, P, T], BF, kind="ExternalOutput")

    with tile.TileContext(nc) as tc:
        with (
            tc.tile_pool(name="const", bufs=1) as cpool,
            tc.tile_pool(name="acts", bufs=1) as apool,
        ):
            ident_f = cpool.tile([P, P], FP)
            make_identity(nc, ident_f[:])
            ident = cpool.tile([P, P], BF)
            nc.vector.tensor_copy(ident[:], ident_f[:])
            cb = cpool.tile([P, 9], FP)
            nc.sync.dma_start(cb[:], cb_d[:])
            sel = cpool.tile([P, NM, P], BF)
            nc.sync.dma_start(sel[:], sel_d[:])
            eps = cpool.tile([P, 1], FP)
            nc.vector.memset(eps[:], LN_EPS)

            rnT = apool.tile([P, NKC, T], BF, tag="rnT")
            qT = apool.tile([P, NM, T], BF, tag="qT")
            # Scores stationaries, zero-padded so every matmul is full K=128:
            # kTa rows 0-63 = head-a k rows (rows 64-127 zero), kTb vice versa.
            kTa = apool.tile([P, NM, TC], BF, tag="kTa")
            kTb = apool.tile([P, NM, TC], BF, tag="kTb")
            vv = apool.tile([P, NJ, HL * (HD + 1)], BF, tag="vv")
            attnU = apool.tile([P, NM, T], BF, tag="attnU")
            attnT = apool.tile([P, NM, T], BF, tag="attnT")
            # denominator rows live at 32-aligned partitions (BIR requires
            # engine APs to start on partition multiples of 32):
            # row(i, hh) -> partition 32*(2*(i%2)+hh), column half i//2
            denp = apool.tile([P, 2 * T], FP, tag="denp")
            recipp = apool.tile([P, 2 * T], FP, tag="recipp")
            recipb = apool.tile([P, 2 * T], BF, tag="recipb")

            # one-time zero/one fills (DVE, overlapped with initial DMAs)
            nc.vector.memset(kTa[64:128, :, :], 0.0)
            nc.vector.memset(kTb[0:64, :, :], 0.0)
            nc.vector.memset(denp[:], 1.0)
            # ones column of v' (denominator rows)
            nc.vector.memset(
                vv[:].rearrange("p j (h c) -> p (j h) c", c=HD + 1)[:, :, HD : HD + 1],
                1.0,
            )
            # preload the Exp activation table before the attention phase
            dummy = cpool.tile([P, 1], BF)

            with (
                tc.tile_pool(name="psmm", bufs=2, space="PSUM") as psmm,
                tc.tile_pool(name="ctx", bufs=2) as ctxpool,
            ):
                # ---- phase A1: rownorm + transpose ----
                with (
                    tc.tile_pool(name="xrn", bufs=10) as xpool,
                    tc.tile_pool(name="xst", bufs=6) as spool,
                    tc.tile_pool(name="sqp", bufs=2) as sqpool,
                    tc.tile_pool(name="wqp", bufs=1) as wqpool,
                    tc.tile_pool(name="pstr", bufs=2, space="PSUM") as pstr,
                ):
                    xts = {}
                    for t in range(NT):
                        xts[t] = xpool.tile([P, C], BF, tag="xt", name=f"xt{t}")
                        nc.sync.dma_start(xts[t][:], x_d[t])
                    wq = wqpool.tile([P, NKC, CL], BF, tag="wq")
                    nc.sync.dma_start(wq[:], wqT_d[:])
                    ctxT0 = ctxpool.tile([P, NKC, TC // 2], BF, tag="ctxT", name="ctxT0")
                    for k in range(NKC):
                        nc.sync.dma_start(ctxT0[:, k, :], ctxT_d[:, k, 0 : TC // 2])
                    for t in range(NT):
                        xt = xts[t]
                        nmu = spool.tile([P, 1], FP, tag="nmu")
                        nc.vector.reduce_sum(nmu[:], xt[:], axis=mybir.AxisListType.X)
                        nc.scalar.mul(nmu[:], nmu[:], -1.0 / C)
                        sq = sqpool.tile([P, C], BF, tag="sq")
                        ex2 = spool.tile([P, 1], FP, tag="ex2")
                        nc.scalar.activation(
                            sq[:], xt[:], mybir.ActivationFunctionType.Square,
                            accum_out=ex2[:],
                        )
                        var = spool.tile([P, 1], FP, tag="var")
                        nc.scalar.mul(ex2[:], ex2[:], 1.0 / C)
                        mu2 = spool.tile([P, 1], FP, tag="mu2")
                        nc.vector.tensor_mul(mu2[:], nmu[:], nmu[:])
                        nc.vector.tensor_sub(var[:], ex2[:], mu2[:])
                        std = spool.tile([P, 1], FP, tag="std")
                        nc.scalar.activation(
                            std[:], var[:], mybir.ActivationFunctionType.Sqrt,
                            bias=eps[:],
                        )
                        inv = spool.tile([P, 1], FP, tag="inv")
                        nc.vector.reciprocal(inv[:], std[:])
                        rn = xpool.tile([P, C], BF, tag="rn")
                        nc.vector.scalar_tensor_tensor(
                            out=rn[:], in0=xt[:], scalar=nmu[:],
                            in1=inv[:].to_broadcast((P, C)),
                            op0=mybir.AluOpType.add, op1=mybir.AluOpType.mult,
                        )
                        # 4 transposes per PSUM tile, one evacuation copy each
                        for c4 in range(2):
                            pt = pstr.tile([P, 512], BF, tag="ptr")
                            for cc in range(4):
                                c = 4 * c4 + cc
                                nc.tensor.transpose(
                                    pt[:, cc * P : (cc + 1) * P],
                                    rn[:, c * P : (c + 1) * P],
                                    ident[:],
                                )
                            nc.vector.tensor_copy(
                                rnT[:, 4 * c4 : 4 * c4 + 4, t * P : (t + 1) * P],
                                pt[:].rearrange("p (c q) -> p c q", q=P),
                            )

                    # ---- q projection: qT[m] += wq[k,m]^T @ rnT[k] ----
                    for m in range(NM):
                        ps = psmm.tile([P, T], FP, tag="mm")
                        for n in range(2):
                            ns = slice(n * 512, (n + 1) * 512)
                            for k in range(NKC):
                                nc.tensor.matmul(
                                    ps[:, ns],
                                    wq[:, k, m * P : (m + 1) * P],
                                    rnT[:, k, ns],
                                    start=(k == 0), stop=(k == NKC - 1),
                                )
                        nc.vector.tensor_scalar(
                            out=qT[:, m, :], in0=ps[:],
                            scalar1=cb[:, m : m + 1], scalar2=cb[:, 8:9],
                            op0=mybir.AluOpType.add, op1=mybir.AluOpType.mult,
                        )
                    # preload exp table after the last Sqrt
                    nc.scalar.activation(
                        dummy[:], eps[:], mybir.ActivationFunctionType.Exp,
                    )

                # ---- k/v projections, context streamed in halves ----
                with (
                    tc.tile_pool(name="wkv", bufs=1) as wpool,
                ):
                    wk = wpool.tile([P, NKC, CL], BF, tag="wk")
                    wv = wpool.tile([P, NKC, CL], BF, tag="wv")
                    nc.sync.dma_start(wk[:], wkT_d[:])
                    nc.sync.dma_start(wv[:], wvT_d[:])
                    for ch in range(2):
                        if ch == 0:
                            ctxT = ctxT0
                        else:
                            ctxT = ctxpool.tile([P, NKC, TC // 2], BF, tag="ctxT", name="ctxT1")
                            for k in range(NKC):
                                nc.sync.dma_start(
                                    ctxT[:, k, :], ctxT_d[:, k, TC // 2 : TC],
                                )
                        hs = slice(ch * (TC // 2), (ch + 1) * (TC // 2))
                        # k projection for this context half
                        for m in range(NM):
                            ps = psmm.tile([P, TC // 2], FP, tag="mm")
                            for n in range(2):
                                ns = slice(n * 512, (n + 1) * 512)
                                for k in range(NKC):
                                    nc.tensor.matmul(
                                        ps[:, ns],
                                        wk[:, k, m * P : (m + 1) * P],
                                        ctxT[:, k, ns],
                                        start=(k == 0), stop=(k == NKC - 1),
                                    )
                            nc.vector.tensor_scalar_add(
                                kTa[0:64, m, hs], ps[0:64, :], cb[0:64, 4 + m : 5 + m],
                            )
                            nc.vector.tensor_scalar_add(
                                kTb[64:128, m, hs], ps[64:128, :], cb[64:128, 4 + m : 5 + m],
                            )
                        # v projection for this context half
                        for jj in range(NJ // 2):
                            j = ch * (NJ // 2) + jj
                            ps = psmm.tile([P, CL], FP, tag="mmv")
                            for k in range(NKC):
                                nc.tensor.matmul(
                                    ps[:],
                                    ctxT[:, k, jj * P : (jj + 1) * P],
                                    wv[:, k, :],
                                    start=(k == 0), stop=(k == NKC - 1),
                                )
                            nc.vector.tensor_copy(
                                vv[:, j, :].rearrange("p (h c) -> p h c", c=HD + 1)[:, :, 0:HD],
                                ps[:].rearrange("p (h c) -> p h c", c=HD),
                            )

            # ---- attention: scores -> exp (ACT/DVE split) -> attn-out ----
            with (
                tc.tile_pool(name="wo", bufs=1) as wopool,
                tc.tile_pool(name="expa", bufs=3) as eapool,
                tc.tile_pool(name="expb", bufs=3) as ebpool,
            ):
                wo = wopool.tile([P, NM, C], BF, tag="wo")
                nc.sync.dma_start(wo[:], woT_d[:])

                with (
                    tc.tile_pool(name="pssc", bufs=2, space="PSUM") as pssc,
                    tc.tile_pool(name="psat", bufs=2, space="PSUM") as psat,
                ):
                    for i in range(NM):
                        ph = {}
                        for hh in range(2):
                            ph[hh] = psat.tile(
                                [HD + 1, T], FP, tag="ph", name=f"ph_{i}_{hh}",
                            )
                        ao_pending = []
                        for j in range(NJ):
                            js = slice(j * P, (j + 1) * P)
                            psc_a = pssc.tile([P, T], FP, tag="sc", name=f"sa{i}_{j}")
                            psc_b = pssc.tile([P, T], FP, tag="sc", name=f"sb{i}_{j}")
                            for n in range(2):
                                ns = slice(n * 512, (n + 1) * 512)
                                nc.tensor.matmul(psc_a[:, ns], kTa[:, i, js],
                                                 qT[:, i, ns],
                                                 start=True, stop=True)
                                nc.tensor.matmul(psc_b[:, ns], kTb[:, i, js],
                                                 qT[:, i, ns],
                                                 start=True, stop=True)
                            et_a = eapool.tile([P, T], BF, tag="eta", name=f"ea{i}_{j}")
                            nc.scalar.activation(
                                et_a[:], psc_a[:], mybir.ActivationFunctionType.Exp,
                            )
                            if j % 2 == 0:
                                et_b = ebpool.tile([P, T], I16, tag="etb", name=f"eb{i}_{j}")
                                nc.vector.tensor_scalar(
                                    out=et_b[:], in0=psc_b[:],
                                    scalar1=EXP_A, scalar2=EXP_B,
                                    op0=mybir.AluOpType.mult, op1=mybir.AluOpType.add,
                                )
                                et_b_bf = et_b[:].bitcast(BF)
                            else:
                                et_b2 = eapool.tile([P, T], BF, tag="eta", name=f"eb{i}_{j}")
                                nc.scalar.activation(
                                    et_b2[:], psc_b[:], mybir.ActivationFunctionType.Exp,
                                )
                                et_b_bf = et_b2[:]
                            ao_pending.append((j, et_a, et_b_bf))
                            if j >= 1:
                                jp, pa, pb = ao_pending.pop(0)
                                vs = vv[:, jp, :].rearrange("p (h c) -> p h c", c=HD + 1)
                                for n in range(2):
                                    ns = slice(n * 512, (n + 1) * 512)
                                    nc.tensor.matmul(
                                        ph[0][:, ns], vs[:, 2 * i, :], pa[:, ns],
                                        start=(jp == 0), stop=(jp == NJ - 1),
                                    )
                                    nc.tensor.matmul(
                                        ph[1][:, ns], vs[:, 2 * i + 1, :],
                                        pb[:, ns],
                                        start=(jp == 0), stop=(jp == NJ - 1),
                                    )
                        jp, pa, pb = ao_pending.pop(0)
                        vs = vv[:, jp, :].rearrange("p (h c) -> p h c", c=HD + 1)
                        for n in range(2):
                            ns = slice(n * 512, (n + 1) * 512)
                            nc.tensor.matmul(ph[0][:, ns], vs[:, 2 * i, :], pa[:, ns],
                                             start=(jp == 0), stop=(jp == NJ - 1))
                            nc.tensor.matmul(ph[1][:, ns], vs[:, 2 * i + 1, :],
                                             pb[:, ns],
                                             start=(jp == 0), stop=(jp == NJ - 1))
                        # evacuate: data rows -> attnU (ACT), denom rows -> denp (DVE)
                        for hh in range(2):
                            nc.scalar.copy(
                                attnU[64 * hh : 64 * hh + 64, i, :], ph[hh][0:64, :],
                            )
                            dp = 32 * (2 * (i % 2) + hh)
                            dc = (i // 2) * T
                            nc.vector.tensor_copy(
                                denp[dp : dp + 1, dc : dc + T], ph[hh][64:65, :],
                            )

                # ---- deferred normalize ----
                with tc.tile_pool(name="psel", bufs=2, space="PSUM") as psel:
                    nc.vector.reciprocal_approx_fast(recipp[:], denp[:])
                    nc.vector.tensor_copy(recipb[:], recipp[:])
                    for i in range(NM):
                        dc = (i // 2) * T
                        rb = psel.tile([P, T], FP, tag="rb")
                        for n in range(2):
                            ns = slice(n * 512, (n + 1) * 512)
                            nc.tensor.matmul(
                                rb[:, ns], sel[:, i, :],
                                recipb[:, dc + n * 512 : dc + (n + 1) * 512],
                                start=True, stop=True,
                            )
                        nc.vector.tensor_mul(
                            attnT[:, i, :], attnU[:, i, :], rb[:],
                        )

                # ---- out-proj partials ----
                with tc.tile_pool(name="psoc", bufs=2, space="PSUM") as psoc:
                    with tc.tile_pool(name="oev", bufs=3) as opool:
                        for m in range(C // P):
                            po = psoc.tile([P, T], FP, tag="oc")
                            for n in range(2):
                                ns = slice(n * 512, (n + 1) * 512)
                                for k2 in range(NM):
                                    nc.tensor.matmul(
                                        po[:, ns],
                                        wo[:, k2, m * P : (m + 1) * P],
                                        attnT[:, k2, ns],
                                        start=(k2 == 0), stop=(k2 == NM - 1),
                                    )
                            ot = opool.tile([P, T], BF, tag="ot")
                            nc.vector.tensor_copy(ot[:], po[:])
                            nc.sync.dma_start(part_d[m], ot[:])

    nc.finalize()
    return nc


_NC_CACHE = {}


def _get_nc():
    if "nc" not in _NC_CACHE:
        _NC_CACHE["nc"] = _build_nc()
    return _NC_CACHE["nc"]


def _quant(w):
    g = np.float32(np.mean(np.abs(w), dtype=np.float64))
    t = np.clip(np.rint(w / (g + np.float32(Q_EPS))), -1.0, 1.0).astype(np.float32)
    return t, g


def _pack_kp(a):
    # [K, M] -> [P, K//P, M] (partition-major chunks)
    k, m = a.shape
    return np.ascontiguousarray(a.reshape(k // P, P, m).transpose(1, 0, 2))


def _bf(a):
    return np.ascontiguousarray(a.astype(ml_dtypes.bfloat16))


def kernel(**inputs):
    global last_exec_time_ns
    x = np.asarray(inputs["x"], dtype=np.float32)
    ctx = np.asarray(inputs["context"], dtype=np.float32)
    Wq = np.asarray(inputs["Wq"], dtype=np.float32)
    Wk = np.asarray(inputs["Wk"], dtype=np.float32)
    Wv = np.asarray(inputs["Wv"], dtype=np.float32)
    Wo = np.asarray(inputs["Wo"], dtype=np.float32)
    bq = np.asarray(inputs["bq"], dtype=np.float32)
    bk = np.asarray(inputs["bk"], dtype=np.float32)
    bv = np.asarray(inputs["bv"], dtype=np.float32)
    bo = np.asarray(inputs["bo"], dtype=np.float32)
    g_ln = np.asarray(inputs["ln_gamma"], dtype=np.float32)
    b_ln = np.asarray(inputs["ln_beta"], dtype=np.float32)

    Tq, gq = _quant(Wq)
    Tk, gk = _quant(Wk)
    Tv, gv = _quant(Wv)
    To, go = _quant(Wo)

    qb_full = (bq + b_ln @ (gq * Tq).T) / gq          # [C]
    scale = np.float32(gq * gk * SCALE)
    host_bias = bo + bv @ (go * To).T                 # [C]

    # select matrices for the denominator broadcast: recipp partition
    # 32*(2*(i%2)+hh) feeds partitions [64*hh, 64*hh+64) of attnT chunk i
    selm = np.zeros((P, NM, P), dtype=np.float32)
    for i in range(NM):
        selm[32 * (2 * (i % 2)), i, 0:64] = 1.0
        selm[32 * (2 * (i % 2) + 1), i, 64:128] = 1.0

    in_maps = []
    for core in range(NCORES):
        b = core // 2
        g = core % 2
        rows = slice(CL * g, CL * (g + 1))
        wqT = _pack_kp((Tq[rows] * g_ln[None, :]).T)  # [P, 8, 512]
        wkT = _pack_kp(Tk[rows].T)
        wvT = _pack_kp(Tv[rows].T)
        woT = _pack_kp((To[:, rows] * (go * gv)).T)   # [P, 4, 1024]
        cbm = np.zeros((P, 9), dtype=np.float32)
        cbm[:, 0:4] = qb_full[rows].reshape(4, P).T
        cbm[:, 4:8] = (bk[rows] / gk).reshape(4, P).T
        cbm[:, 8] = scale
        in_maps.append({
            "x": _bf(x[b].reshape(T // P, P, C)),
            "ctxT": _bf(_pack_kp(np.ascontiguousarray(ctx[b].T))),
            "wqT": _bf(wqT), "wkT": _bf(wkT), "wvT": _bf(wvT), "woT": _bf(woT),
            "cb": cbm,
            "sel": _bf(selm),
        })

    nc = _get_nc()
    trace = os.environ.get("KERNEL_TRACE", "0") == "1"
    res = run_bass_kernel_spmd(nc, in_maps, list(range(NCORES)), trace=trace)
    last_exec_time_ns = res.exec_time_ns

    out = np.empty((B, T, C), dtype=np.float32)
    for b in range(B):
        p0 = res.results[2 * b]["partial"].astype(np.float32).reshape(C, T)
        p1 = res.results[2 * b + 1]["partial"].astype(np.float32).reshape(C, T)
        out[b] = x[b] + p0.T + p1.T + host_bias[None, :]
    return out
